# revision 24
# baseline (speedup 1.0000x reference)
"""Bass/Trainium2 kernel for a BiGRU encoder-decoder (B=1024, S=128, T=32, H=512, IN=3).

Sharding: data-parallel over batch across 8 NeuronCores (128 rows/core);
all weights replicated; sequential scans local to each core.

Scan/decoder phases use a "stationary-h" matmul orientation: the hidden
state h (as [feat, batch] ktiles) is the PE stationary operand, reused
across three N=512 matmuls per ktile whose moving operand is W^T
([feat, gates]); gates accumulate in PSUM as [batch, gate]. This cuts
weight-load (LDWEIGHTS) traffic ~10x vs streaming h against per-(k,g)
stationary weight blocks. h is kept in both layouts: gate math runs in
[batch, feat]; a 4-matmul identity transpose rebuilds [feat, batch] for
the next step's stationary (and the DRAM out0 feed of the layer-1 gemm).
The layer-1 input gemm (phase 1) keeps the classic weight-stationary
orientation (N=512 over 4 timesteps). Matmul operands fp16, accumulation
and gate math fp32/fp16 as before.
"""

import os
import sys
import numpy as np
from contextlib import ExitStack

sys.path.insert(0, "/opt/trn_rl_repo")
import concourse.bass as bass  # noqa: E402,F401
import concourse.tile as tile  # noqa: E402
from concourse import bacc, mybir  # noqa: E402
from concourse.bass_utils import run_bass_kernel_spmd  # noqa: E402

FP16 = mybir.dt.float16
FP32 = mybir.dt.float32
AF = mybir.ActivationFunctionType

B, S, T, H, IN = 1024, 128, 32, 512, 3
NCORE = 8
BC = B // NCORE          # 128 batch rows per core
KT = H // 128            # 4 hidden k-tiles
GT = (3 * H) // 128      # 12 gate tiles (r 0-3, z 4-7, n 8-11)


# ----------------------------------------------------------------- host packing

def _pack_st(W, nk, ng):
    """Stationary pack of W [ng*128, nk*128] -> [128, nk*ng*128] fp16.

    Column block (k*ng + g)*128 holds lhsT for (ktile k, gtile g):
    lhsT[kf, gc] = W[g*128+gc, k*128+kf]. (phase-1 gemm only)
    """
    t = W.reshape(ng, 128, nk, 128).transpose(2, 3, 0, 1)  # [k, kf, g, gc]
    return np.ascontiguousarray(
        t.transpose(1, 0, 2, 3).reshape(128, nk * ng * 128)
    ).astype(np.float16)


def _pack_whhT(W):
    """Moving-operand pack of Whh [3H, H] -> [128, KT*3H] fp16.

    Column block k*1536..(k+1)*1536 = Whh[:, k*128:(k+1)*128].T, i.e.
    [kf, g] with g ordered r(0:512) z(512:1024) n(1024:1536).
    """
    out = np.zeros((128, KT * 3 * H), np.float32)
    for k in range(KT):
        out[:, k * 3 * H:(k + 1) * 3 * H] = W[:, k * 128:(k + 1) * 128].T
    return out.astype(np.float16)


def _pack_wihbT(Wih, bih, bhh):
    """x-stationary moving pack: [4, 2048] fp16.

    Rows 0-2 = Wih^T (IN=3 input feats), row 3 = bias.
    Cols: r(0:512)=Wih_r^T,b_r sum | z | ni(1024:1536)=Wih_n^T,bih_n |
    nh(1536:2048)= zeros, bhh_n.
    """
    out = np.zeros((4, 4 * H), np.float32)
    out[0:IN, 0:512] = Wih[0:512, :].T
    out[0:IN, 512:1024] = Wih[512:1024, :].T
    out[0:IN, 1024:1536] = Wih[1024:1536, :].T
    bsum = bih + bhh
    out[3, 0:512] = bsum[0:512]
    out[3, 512:1024] = bsum[512:1024]
    out[3, 1024:1536] = bih[1024:1536]
    out[3, 1536:2048] = bhh[1024:1536]
    return out.astype(np.float16)


def _pack_wih_hT(Wih):
    """hidden-input moving pack for decoder layers 1-3: [128, KT*3H] fp16."""
    return _pack_whhT(Wih)


def _bias_rows(bih, bhh):
    """[1, 2048] bias row: r,z sums; ni=bih_n; nh=bhh_n."""
    row = np.zeros((1, 4 * H), np.float32)
    bsum = bih + bhh
    row[0, 0:512] = bsum[0:512]
    row[0, 512:1024] = bsum[512:1024]
    row[0, 1024:1536] = bih[1024:1536]
    row[0, 1536:2048] = bhh[1024:1536]
    return row.astype(np.float16)


def _host_pack(inp, s_steps, t_steps):
    """Build replicated weight arrays + per-core input arrays."""
    g = lambda k: np.asarray(inp[k], np.float32)
    w = {}
    for d, tag in ((0, "f"), (1, "b")):
        w[f"w_hhT0{tag}"] = _pack_whhT(g("enc_Whh0")[d])
        w[f"w_ihbT0{tag}"] = _pack_wihbT(
            g("enc_Wih0")[d], g("enc_bih0")[d], g("enc_bhh0")[d])
        w[f"w_hhT1{tag}"] = _pack_whhT(g("enc_Whh1")[d])
        w[f"b_row1{tag}"] = _bias_rows(g("enc_bih1")[d], g("enc_bhh1")[d])
    # encoder layer 1 input gemm (phase 1, weight-stationary): K = 2H, G = 2*GT
    w_all = np.concatenate([g("enc_Wih1")[0], g("enc_Wih1")[1]], axis=0)  # [2*3H, 2H]
    w["w_ih1"] = _pack_st(w_all, 2 * KT, 2 * GT)
    bg = np.zeros((128, 2 * GT), np.float32)
    for d in range(2):
        bsum = g("enc_bih1")[d] + g("enc_bhh1")[d]
        for j in range(8):
            bg[:, d * GT + j] = bsum[j * 128:(j + 1) * 128]
        for j in range(4):
            bg[:, d * GT + 8 + j] = g("enc_bih1")[d][1024 + j * 128:1024 + (j + 1) * 128]
    w["b_gemm"] = bg
    w["ident"] = np.eye(128, dtype=np.float16)
    # decoder
    w["w_dihbT0"] = _pack_wihbT(g("dec_Wih0"), g("dec_bih0"), g("dec_bhh0"))
    w["w_dhhT"] = np.concatenate(
        [_pack_whhT(g("dec_Whh0"))]
        + [_pack_whhT(g("dec_Whh123")[i]) for i in range(3)], axis=1)
    w["w_dihT"] = np.concatenate(
        [_pack_wih_hT(g("dec_Wih123")[i]) for i in range(3)], axis=1)
    w["w_dbias"] = np.concatenate(
        [_bias_rows(g("dec_bih123")[i], g("dec_bhh123")[i]) for i in range(3)],
        axis=1)  # [1, 3*2048]
    lw = np.zeros((128, KT), np.float32)
    for k in range(KT):
        lw[:, k] = g("lin_W")[0, k * 128:(k + 1) * 128]
    w["w_lin"] = lw.astype(np.float16)
    lin_b = float(np.asarray(inp["lin_b"]).reshape(-1)[0])

    x = np.asarray(inp["x"], np.float32)  # [B, S, IN]
    per_core = []
    for c in range(NCORE):
        xc = x[c * BC:(c + 1) * BC]                      # [BC, S, IN]
        xt = xc.transpose(2, 1, 0)[:, :s_steps, :]       # [IN, s, BC]
        xa = np.zeros((4, s_steps, BC), np.float32)
        xa[0:IN] = xt
        xa[IN] = 1.0
        m = dict(w)
        m["xa"] = np.ascontiguousarray(xa.reshape(4, s_steps * BC)).astype(np.float16)
        xlast = xc[:, s_steps - 1, :].T                  # [IN, BC]
        m["xd0"] = xlast.astype(np.float16)
        m["xd32"] = np.ascontiguousarray(xlast[0:2].T).astype(np.float32)  # [BC, 2]
        per_core.append(m)
    return per_core, lin_b


# ----------------------------------------------------------------- device build

def build_nc(s_steps=S, t_steps=T, lin_b=0.0, repeat=1):
    nc = bacc.Bacc("TRN2", target_bir_lowering=False, debug=False,
                   num_devices=NCORE)

    dram_in = {}
    for name, shape, dt in [
        ("w_hhT0f", [128, KT * 3 * H], FP16), ("w_hhT0b", [128, KT * 3 * H], FP16),
        ("w_ihbT0f", [4, 4 * H], FP16), ("w_ihbT0b", [4, 4 * H], FP16),
        ("w_hhT1f", [128, KT * 3 * H], FP16), ("w_hhT1b", [128, KT * 3 * H], FP16),
        ("b_row1f", [1, 4 * H], FP16), ("b_row1b", [1, 4 * H], FP16),
        ("w_ih1", [128, 2 * KT * 2 * GT * 128], FP16),
        ("b_gemm", [128, 2 * GT], FP32),
        ("ident", [128, 128], FP16),
        ("w_dihbT0", [4, 4 * H], FP16),
        ("w_dhhT", [128, 4 * KT * 3 * H], FP16),
        ("w_dihT", [128, 3 * KT * 3 * H], FP16),
        ("w_dbias", [1, 3 * 4 * H], FP16),
        ("w_lin", [128, KT], FP16),
        ("xa", [4, s_steps * BC], FP16),
        ("xd0", [3, BC], FP16),
        ("xd32", [BC, 2], FP32),
    ]:
        dram_in[name] = nc.dram_tensor(name, shape, dt, kind="ExternalInput").ap()

    y_dram = nc.dram_tensor("y", [BC, t_steps], FP32, kind="ExternalOutput").ap()

    NCH = s_steps // 4  # gemm chunks (4 timesteps each)

    with tile.TileContext(nc) as tc, ExitStack() as ctx:
        const = ctx.enter_context(tc.tile_pool(name="const", bufs=1))
        hp = ctx.enter_context(tc.tile_pool(name="hstate", bufs=2))
        dram = ctx.enter_context(tc.tile_pool(name="dram", bufs=1, space="DRAM"))

        ones1 = const.tile([1, BC], FP16, tag="ones1")
        nc.vector.memset(ones1[:], 1.0)
        ones512 = const.tile([128, 512], FP16, tag="ones512")
        nc.vector.memset(ones512[:], 1.0)
        linb_sb = const.tile([1, 1], FP32, tag="linb_sb")
        nc.vector.memset(linb_sb[:], float(lin_b))
        y_sb = const.tile([128, t_steps], FP32, tag="y_sb")

        out0_d = [dram.tile([s_steps, 128, KT * 128], FP16, tag=f"out0_{d}",
                            name=f"out0_{d}") for d in range(2)]
        gi1_d = [dram.tile([s_steps, 128, GT * 128], FP16, tag=f"gi1_{d}",
                           name=f"gi1_{d}") for d in range(2)]

        def load_const(name, pool):
            ap = dram_in[name]
            t = pool.tile(list(ap.shape), ap.dtype, tag=name)
            nc.sync.dma_start(t[:], ap[:])
            return t

        def gates_bf(nc_, gw, pr, pz, pni, pnh, h_bf_prev, tag, htag=None):
            """[batch, gate]-layout gate math; returns new h_bf (fp16).

            h' = (1-z)*n + z*h; z*h and (1-z) are computed while the tanh
            runs so only two DVE ops trail the tanh on the critical chain.
            """
            htag = htag or tag
            r16 = gw.tile([128, 512], FP16, tag=f"r{tag}")
            nc_.scalar.activation(r16[:], pr[:], AF.Sigmoid)
            z16 = gw.tile([128, 512], FP16, tag=f"z{tag}")
            nc_.scalar.activation(z16[:], pz[:], AF.Sigmoid)
            t2 = gw.tile([128, 512], FP16, tag=f"t2{tag}")
            nc_.vector.tensor_mul(t2[:], r16[:], pnh[:])
            t3 = gw.tile([128, 512], FP16, tag=f"t3{tag}")
            nc_.vector.tensor_add(t3[:], t2[:], pni[:])
            n16 = gw.tile([128, 512], FP16, tag=f"n{tag}")
            nc_.scalar.activation(n16[:], t3[:], AF.Tanh)
            q16 = gw.tile([128, 512], FP16, tag=f"q{tag}")
            nc_.vector.tensor_mul(q16[:], z16[:], h_bf_prev[:])
            oz16 = gw.tile([128, 512], FP16, tag=f"oz{tag}")
            nc_.vector.tensor_sub(oz16[:], ones512[:], z16[:])
            m16 = gw.tile([128, 512], FP16, tag=f"m{tag}")
            nc_.vector.tensor_mul(m16[:], oz16[:], n16[:])
            h_bf = hp.tile([128, 512], FP16, tag=f"hbf{htag}")
            nc_.vector.tensor_add(h_bf[:], m16[:], q16[:])
            return h_bf

        def transpose_h(nc_, psp, ident, h_bf, tag, bufs=1):
            """4 identity matmuls: h_bf [b, feat] -> psum [feat, b] slices."""
            pT = psp.tile([128, 512], FP32, tag="pT", bufs=bufs)
            for k in range(KT):
                nc_.tensor.matmul(pT[:, k * 128:(k + 1) * 128],
                                  h_bf[:, k * 128:(k + 1) * 128], ident[:],
                                  start=(k == 0), stop=(k == KT - 1))
            h_fb = hp.tile([128, 512], FP16, tag=f"hfb{tag}")
            nc_.scalar.activation(h_fb[:], pT[:], AF.Identity)
            return h_fb

        def scan_mms(nc_, psp, wh, h_fb_d):
            """hh matmuls for one scan step-dir: pr/pz k-major first
            (single-buffered — their WAR frees earliest via the sigmoids),
            pnh k-loop last; pnh/pni double-buffered."""
            pr = psp.tile([128, 512], FP32, tag="pr", bufs=1)
            pz = psp.tile([128, 512], FP32, tag="pz", bufs=1)
            for k in range(KT):
                hk = h_fb_d[:, k * 128:(k + 1) * 128]
                st = (k == 0)
                nc_.tensor.matmul(pr[:], hk, wh[:, k * 1536:k * 1536 + 512],
                                  start=st, stop=False)
                nc_.tensor.matmul(pz[:], hk,
                                  wh[:, k * 1536 + 512:k * 1536 + 1024],
                                  start=st, stop=False)
            pnh = psp.tile([128, 512], FP32, tag="pnh", bufs=2)
            for k in range(KT):
                hk = h_fb_d[:, k * 128:(k + 1) * 128]
                nc_.tensor.matmul(pnh[:], hk,
                                  wh[:, k * 1536 + 1024:k * 1536 + 1536],
                                  start=(k == 0), stop=False)
            pni = psp.tile([128, 512], FP32, tag="pni", bufs=2)
            return pr, pz, pni, pnh

        NPH = int(os.environ.get("BGRU_PHASES", "4"))

        def emit_phases(sfx):
          # =================== phase 0: encoder layer-0 bidirectional scan
          if NPH >= 1:
            with tc.tile_pool(name="p0w" + sfx, bufs=1) as p0w, \
                 tc.tile_pool(name="gw0" + sfx, bufs=2) as gw0, \
                 tc.tile_pool(name="psum0" + sfx, bufs=1, space="PSUM") as ps0:
              whhT = [load_const("w_hhT0f", p0w), load_const("w_hhT0b", p0w)]
              wihbT = [load_const("w_ihbT0f", p0w), load_const("w_ihbT0b", p0w)]
              xa = load_const("xa", p0w)
              ident = load_const("ident", p0w)

              h_bf = [None, None]
              h_fb = [None, None]
              for d in range(2):
                  h_bf[d] = hp.tile([128, 512], FP16, tag=f"hbf_e0{d}",
                                    name=f"hbf_e0{d}")
                  nc.vector.memset(h_bf[d][:], 0.0)
                  h_fb[d] = hp.tile([128, 512], FP16, tag=f"hfb_e0{d}",
                                    name=f"hfb_e0{d}")
                  nc.vector.memset(h_fb[d][:], 0.0)

              def p0_mm(d, step):
                  tt = step if d == 0 else s_steps - 1 - step
                  # h-independent ih/bias matmuls OPEN each bank group so the
                  # block starts without waiting on the new h's transpose copy
                  xst = xa[0:4, tt * BC:(tt + 1) * BC]
                  wi = wihbT[d]
                  wh = whhT[d]
                  pr = ps0.tile([128, 512], FP32, tag="pr", bufs=1)
                  pz = ps0.tile([128, 512], FP32, tag="pz", bufs=1)
                  pnh = ps0.tile([128, 512], FP32, tag="pnh", bufs=2)
                  pni = ps0.tile([128, 512], FP32, tag="pni", bufs=2)
                  nc.tensor.matmul(pr[:], xst, wi[0:4, 0:512],
                                   start=True, stop=False)
                  nc.tensor.matmul(pz[:], xst, wi[0:4, 512:1024],
                                   start=True, stop=False)
                  nc.tensor.matmul(pnh[:], xst, wi[0:4, 1536:2048],
                                   start=True, stop=False)
                  nc.tensor.matmul(pni[:], xst, wi[0:4, 1024:1536],
                                   start=True, stop=True)
                  for k in range(KT):
                      hk = h_fb[d][:, k * 128:(k + 1) * 128]
                      nc.tensor.matmul(pr[:], hk, wh[:, k * 1536:k * 1536 + 512],
                                       start=False, stop=(k == KT - 1))
                  for k in range(KT):
                      hk = h_fb[d][:, k * 128:(k + 1) * 128]
                      nc.tensor.matmul(pnh[:], hk,
                                       wh[:, k * 1536 + 1024:k * 1536 + 1536],
                                       start=False, stop=(k == KT - 1))
                  for k in range(KT):
                      hk = h_fb[d][:, k * 128:(k + 1) * 128]
                      nc.tensor.matmul(pz[:], hk,
                                       wh[:, k * 1536 + 512:k * 1536 + 1024],
                                       start=False, stop=(k == KT - 1))
                  return (pr, pz, pni, pnh)

              def p0_gates(d, step, ps):
                  tt = step if d == 0 else s_steps - 1 - step
                  h_bf[d] = gates_bf(nc, gw0, *ps, h_bf[d], f"e0{d}")
                  h_fb[d] = transpose_h(nc, ps0, ident, h_bf[d], f"e0{d}",
                                        bufs=2)
                  nc.sync.dma_start(out0_d[d][tt], h_fb[d][:])

              # dir-skewed pipeline: gates/T of d1(s-1) sit between d0(s)'s
              # and d1(s)'s matmul blocks so PE never waits on a gate chain.
              ps_cur = [None, None]
              for step in range(s_steps):
                  ps_cur[0] = p0_mm(0, step)
                  if step > 0:
                      p0_gates(1, step - 1, ps_cur[1])
                  ps_cur[1] = p0_mm(1, step)
                  p0_gates(0, step, ps_cur[0])
              p0_gates(1, s_steps - 1, ps_cur[1])
              h_dec0 = [h_fb[0], h_fb[1]]  # (h0f, h0b) [feat, b]
              h_dec0_bf = [h_bf[0], h_bf[1]]

          # =================== phase 1: in1 @ Wih1.T big gemm (4 timesteps/chunk)
          if NPH >= 2:
            with tc.tile_pool(name="p1w" + sfx, bufs=1) as p1w, \
                 tc.tile_pool(name="gmv" + sfx, bufs=3) as gmv, \
                 tc.tile_pool(name="gst" + sfx, bufs=6) as gst, \
                 tc.tile_pool(name="psumg" + sfx, bufs=8, space="PSUM") as psg:
              wih1 = load_const("w_ih1", p1w)
              bgem = load_const("b_gemm", p1w)

              order = []
              lo, hi = 0, NCH - 1
              while lo <= hi:
                  order.append(lo)
                  if hi != lo:
                      order.append(hi)
                  lo, hi = lo + 1, hi - 1

              for c in order:
                  mv = []
                  for d in range(2):
                      for j in range(KT):
                          m = gmv.tile([128, 512], FP16, tag=f"mv{d}{j}")
                          src = out0_d[d][4 * c:4 * c + 4, :, j * 128:(j + 1) * 128]
                          nc.sync.dma_start(
                              m[:].rearrange("p (t b) -> p t b", t=4),
                              src.rearrange("t p b -> p t b"))
                          mv.append(m)
                  for G in range(2 * GT):
                      q = psg.tile([128, 512], FP32, tag="gq")
                      for k in range(2 * KT):
                          nc.tensor.matmul(
                              q[:],
                              wih1[:, (k * 2 * GT + G) * 128:(k * 2 * GT + G + 1) * 128],
                              mv[k][:], start=(k == 0), stop=(k == 2 * KT - 1))
                      gsb = gst.tile([128, 512], FP16, tag="gsb")
                      nc.scalar.activation(gsb[:], q[:], AF.Identity,
                                           bias=bgem[:, G:G + 1])
                      d_, gl = (0, G) if G < GT else (1, G - GT)
                      dst = gi1_d[d_][4 * c:4 * c + 4, :, gl * 128:(gl + 1) * 128]
                      nc.sync.dma_start(dst.rearrange("t p b -> p t b"),
                                        gsb[:].rearrange("p (t b) -> p t b", t=4))

          # =================== phase 2: encoder layer-1 bidirectional scan
          if NPH >= 3:
            with tc.tile_pool(name="p2w" + sfx, bufs=1) as p2w, \
                 tc.tile_pool(name="gw2" + sfx, bufs=2) as gw2, \
                 tc.tile_pool(name="gild" + sfx, bufs=3) as gild, \
                 tc.tile_pool(name="psum1" + sfx, bufs=1, space="PSUM") as ps1:
              whhT1 = [load_const("w_hhT1f", p2w), load_const("w_hhT1b", p2w)]
              brow1 = [load_const("b_row1f", p2w), load_const("b_row1b", p2w)]
              ident = load_const("ident", p2w)

              h_bf = [None, None]
              h_fb = [None, None]
              for d in range(2):
                  h_bf[d] = hp.tile([128, 512], FP16, tag=f"hbf_e1{d}",
                                    name=f"hbf_e1{d}")
                  nc.vector.memset(h_bf[d][:], 0.0)
                  h_fb[d] = hp.tile([128, 512], FP16, tag=f"hfb_e1{d}",
                                    name=f"hfb_e1{d}")
                  nc.vector.memset(h_fb[d][:], 0.0)

              def p2_mm(d, step):
                  tt = step if d == 0 else s_steps - 1 - step
                  gld = gild.tile([128, GT * 128], FP16, tag=f"gi{d}")
                  nc.sync.dma_start(gld[:], gi1_d[d][tt])
                  wh = whhT1[d]
                  pr = ps1.tile([128, 512], FP32, tag="pr", bufs=1)
                  pz = ps1.tile([128, 512], FP32, tag="pz", bufs=1)
                  pnh = ps1.tile([128, 512], FP32, tag="pnh", bufs=2)
                  pni = ps1.tile([128, 512], FP32, tag="pni", bufs=2)
                  # h-independent openers: gi1 injections + nh bias
                  for m in range(4):
                      nc.tensor.matmul(pr[:, m * 128:(m + 1) * 128],
                                       gld[:, m * 128:(m + 1) * 128], ident[:],
                                       start=(m == 0), stop=False)
                  for m in range(4):
                      nc.tensor.matmul(pz[:, m * 128:(m + 1) * 128],
                                       gld[:, (4 + m) * 128:(5 + m) * 128],
                                       ident[:], start=(m == 0), stop=False)
                  nc.tensor.matmul(pnh[:], ones1[:], brow1[d][0:1, 1536:2048],
                                   start=True, stop=False)
                  for m in range(4):
                      nc.tensor.matmul(pni[:, m * 128:(m + 1) * 128],
                                       gld[:, (8 + m) * 128:(9 + m) * 128],
                                       ident[:], start=(m == 0), stop=(m == 3))
                  for k in range(KT):
                      hk = h_fb[d][:, k * 128:(k + 1) * 128]
                      nc.tensor.matmul(pr[:], hk, wh[:, k * 1536:k * 1536 + 512],
                                       start=False, stop=(k == KT - 1))
                  for k in range(KT):
                      hk = h_fb[d][:, k * 128:(k + 1) * 128]
                      nc.tensor.matmul(pnh[:], hk,
                                       wh[:, k * 1536 + 1024:k * 1536 + 1536],
                                       start=False, stop=(k == KT - 1))
                  for k in range(KT):
                      hk = h_fb[d][:, k * 128:(k + 1) * 128]
                      nc.tensor.matmul(pz[:], hk,
                                       wh[:, k * 1536 + 512:k * 1536 + 1024],
                                       start=False, stop=(k == KT - 1))
                  return (pr, pz, pni, pnh)

              def p2_gates(d, ps):
                  h_bf[d] = gates_bf(nc, gw2, *ps, h_bf[d], f"e1{d}")
                  h_fb[d] = transpose_h(nc, ps1, ident, h_bf[d], f"e1{d}",
                                        bufs=2)

              ps_cur = [None, None]
              for step in range(s_steps):
                  ps_cur[0] = p2_mm(0, step)
                  if step > 0:
                      p2_gates(1, ps_cur[1])
                  ps_cur[1] = p2_mm(1, step)
                  p2_gates(0, ps_cur[0])
              p2_gates(1, ps_cur[1])
              h_dec1 = [h_fb[0], h_fb[1]]
              h_dec1_bf = [h_bf[0], h_bf[1]]

          # =================== phase 3: 4-layer decoder, autoregressive
          if NPH >= 4:
            with tc.tile_pool(name="p3w" + sfx, bufs=1) as p3w, \
                 tc.tile_pool(name="gw3" + sfx, bufs=2) as gw3, \
                 tc.tile_pool(name="dx" + sfx, bufs=2) as dx, \
                 tc.tile_pool(name="psumd" + sfx, bufs=1, space="PSUM") as psd:
              wdhhT = load_const("w_dhhT", p3w)
              wdihT = load_const("w_dihT", p3w)
              wdbias = load_const("w_dbias", p3w)
              wdihbT0 = load_const("w_dihbT0", p3w)
              wlin = load_const("w_lin", p3w)
              ident = load_const("ident", p3w)

              # x_aug stationary [4, BC]: rows x0,x1,x2,ones
              xst = dx.tile([4, BC], FP16, tag="xst", bufs=2)
              nc.sync.dma_start(xst[0:3, :], dram_in["xd0"][:])
              nc.sync.dma_start(xst[3:4, :], ones1[:])
              d01 = dx.tile([128, 2], FP32, tag="d01", bufs=1)
              nc.sync.dma_start(d01[:], dram_in["xd32"][:])
              din = [d01[:, 0:1], d01[:, 1:2]]

              hfbL = [h_dec0[0], h_dec0[1], h_dec1[0], h_dec1[1]]
              hbfL = [h_dec0_bf[0], h_dec0_bf[1], h_dec1_bf[0], h_dec1_bf[1]]

              def open_hh(L):
                  """Alloc pr/pz/pnh banks, emit hh matmuls (no stop)."""
                  pr = psd.tile([128, 512], FP32, tag="pr", bufs=2)
                  pz = psd.tile([128, 512], FP32, tag="pz", bufs=2)
                  wh = wdhhT[:, L * KT * 1536:]
                  for k in range(KT):
                      hk = hfbL[L][:, k * 128:(k + 1) * 128]
                      st = (k == 0)
                      nc.tensor.matmul(pr[:], hk, wh[:, k * 1536:k * 1536 + 512],
                                       start=st, stop=False)
                      nc.tensor.matmul(pz[:], hk,
                                       wh[:, k * 1536 + 512:k * 1536 + 1024],
                                       start=st, stop=False)
                  pnh = psd.tile([128, 512], FP32, tag="pnh", bufs=2)
                  for k in range(KT):
                      hk = hfbL[L][:, k * 128:(k + 1) * 128]
                      nc.tensor.matmul(pnh[:], hk,
                                       wh[:, k * 1536 + 1024:k * 1536 + 1536],
                                       start=(k == 0), stop=False)
                  return [pr, pz, None, pnh]

              def close_ih(L, ps, below_fb):
                  pr, pz, _, pnh = ps
                  pni = psd.tile([128, 512], FP32, tag="pni", bufs=1)
                  ps[2] = pni
                  if L == 0:
                      wi = wdihbT0
                      nc.tensor.matmul(pr[:], xst[0:4, :], wi[0:4, 0:512],
                                       start=False, stop=True)
                      nc.tensor.matmul(pz[:], xst[0:4, :], wi[0:4, 512:1024],
                                       start=False, stop=True)
                      nc.tensor.matmul(pnh[:], xst[0:4, :], wi[0:4, 1536:2048],
                                       start=False, stop=True)
                      nc.tensor.matmul(pni[:], xst[0:4, :], wi[0:4, 1024:1536],
                                       start=True, stop=True)
                  else:
                      # below-independent bias openers first, then the
                      # below_fb-dependent k-loops
                      wi = wdihT[:, (L - 1) * KT * 1536:]
                      br = wdbias[0:1, (L - 1) * 2048:]
                      nc.tensor.matmul(pr[:], ones1[:], br[0:1, 0:512],
                                       start=False, stop=False)
                      nc.tensor.matmul(pz[:], ones1[:], br[0:1, 512:1024],
                                       start=False, stop=False)
                      nc.tensor.matmul(pnh[:], ones1[:], br[0:1, 1536:2048],
                                       start=False, stop=True)
                      nc.tensor.matmul(pni[:], ones1[:], br[0:1, 1024:1536],
                                       start=True, stop=False)
                      for k in range(KT):
                          bk = below_fb[:, k * 128:(k + 1) * 128]
                          nc.tensor.matmul(pr[:], bk, wi[:, k * 1536:k * 1536 + 512],
                                           start=False, stop=(k == KT - 1))
                      for k in range(KT):
                          bk = below_fb[:, k * 128:(k + 1) * 128]
                          nc.tensor.matmul(pni[:], bk,
                                           wi[:, k * 1536 + 1024:k * 1536 + 1536],
                                           start=False, stop=(k == KT - 1))
                      for k in range(KT):
                          bk = below_fb[:, k * 128:(k + 1) * 128]
                          nc.tensor.matmul(pz[:], bk,
                                           wi[:, k * 1536 + 512:k * 1536 + 1024],
                                           start=False, stop=(k == KT - 1))

              xnew_prev = None
              for t in range(t_steps):
                  # software-pipelined: hh matmuls of layers L+1/L+2 are
                  # emitted before layer L's input-dependent matmuls.
                  ps_of = {0: open_hh(0), 1: open_hh(1)}
                  if t > 0:
                      # feedback transpose deferred here so the hh matmuls
                      # above cover the feedback chain's latency
                      pT3 = psd.tile([128, 512], FP32, tag="pT", bufs=1)
                      pxT = pT3[0:4, 0:BC]
                      nc.tensor.matmul(pxT, xnew_prev[:, 0:4], ident[:],
                                       start=True, stop=True)
                      xst = dx.tile([4, BC], FP16, tag="xst", bufs=2)
                      nc.scalar.activation(xst[:], pxT, AF.Identity)
                  below_fb = None
                  for L in range(4):
                      close_ih(L, ps_of[L], below_fb)
                      if L + 2 < 4:
                          ps_of[L + 2] = open_hh(L + 2)
                      hbfL[L] = gates_bf(nc, gw3, *ps_of[L], hbfL[L], "dd", htag=f"d{L}")
                      hfbL[L] = transpose_h(nc, psd, ident, hbfL[L], f"d{L}",
                                            bufs=1)
                      below_fb = hfbL[L]

                  # output projection + feedback ([b, 1] column layout)
                  pT2 = psd.tile([128, 512], FP32, tag="pT", bufs=1)
                  pout = pT2[:, 0:1]
                  for k in range(KT):
                      nc.tensor.matmul(pout,
                                       below_fb[:, k * 128:(k + 1) * 128],
                                       wlin[:, k:k + 1],
                                       start=(k == 0), stop=(k == KT - 1))
                  ysl = y_sb[:, t:t + 1]
                  nc.vector.tensor_scalar_add(ysl, pout, float(lin_b))
                  if t + 1 < t_steps:
                      xnew = dx.tile([128, 4], FP16, tag="xnew")
                      s0 = dx.tile([128, 1], FP32, tag="s0")
                      nc.vector.tensor_copy(s0[:], ysl)
                      s1 = dx.tile([128, 1], FP32, tag="s1")
                      nc.vector.tensor_sub(s1[:], din[0], s0[:])
                      s2 = dx.tile([128, 1], FP32, tag="s2")
                      nc.vector.tensor_sub(s2[:], din[1], s1[:])
                      nc.vector.tensor_copy(xnew[:, 0:1], s0[:])
                      nc.vector.tensor_copy(xnew[:, 1:2], s1[:])
                      nc.vector.tensor_copy(xnew[:, 2:3], s2[:])
                      nc.vector.memset(xnew[:, 3:4], 1.0)
                      xnew_prev = xnew
                      din = [s0[:], s1[:]]

              nc.sync.dma_start(y_dram[:], y_sb[:, 0:t_steps])

        for _rep in range(repeat):
            emit_phases(f"_{_rep}" if _rep else "")

    nc.compile()
    return nc


# ----------------------------------------------------------------- entry point

def kernel(**inputs) -> np.ndarray:
    s_steps = int(os.environ.get("BGRU_S", S))
    t_steps = int(os.environ.get("BGRU_T", 0)) or int(inputs.get("target_len", T))
    per_core, lin_b = _host_pack(inputs, s_steps, t_steps)
    nc = build_nc(s_steps, t_steps, lin_b)
    res = run_bass_kernel_spmd(nc, per_core, list(range(NCORE)))
    out = np.zeros((B, t_steps, 1), np.float32)
    for c in range(NCORE):
        yc = res.results[c]["y"].reshape(BC, t_steps)  # [b, t]
        out[c * BC:(c + 1) * BC, :, 0] = yc
    return out


# revision 44
# speedup vs baseline: 1.1064x; 1.1064x over previous
"""Bass/Trainium2 kernel for a BiGRU encoder-decoder (B=1024, S=128, T=32, H=512, IN=3).

Sharding: data-parallel over batch across 8 NeuronCores (128 rows/core);
all weights replicated; sequential scans local to each core.

Scan/decoder phases use a "stationary-h" matmul orientation: the hidden
state h (as [feat, batch] ktiles) is the PE stationary operand, reused
across three N=512 matmuls per ktile whose moving operand is W^T
([feat, gates]); gates accumulate in PSUM as [batch, gate]. This cuts
weight-load (LDWEIGHTS) traffic ~10x vs streaming h against per-(k,g)
stationary weight blocks. h is kept in both layouts: gate math runs in
[batch, feat]; a 4-matmul identity transpose rebuilds [feat, batch] for
the next step's stationary (and the DRAM out0 feed of the layer-1 gemm).
The layer-1 input gemm (phase 1) keeps the classic weight-stationary
orientation (N=512 over 4 timesteps). Matmul operands fp16, accumulation
and gate math fp32/fp16 as before.
"""

import os
import sys
import numpy as np
from contextlib import ExitStack

sys.path.insert(0, "/opt/trn_rl_repo")
import concourse.bass as bass  # noqa: E402,F401
import concourse.tile as tile  # noqa: E402
from concourse import bacc, mybir  # noqa: E402
from concourse.bass_utils import run_bass_kernel_spmd  # noqa: E402

FP16 = mybir.dt.float16
FP32 = mybir.dt.float32
FP8 = mybir.dt.float8e4
NP8 = mybir.dt.np(FP8)
W8SCALE = 16.0  # fp8 weight pre-scale (undone in the gemm output copy)
AF = mybir.ActivationFunctionType

B, S, T, H, IN = 1024, 128, 32, 512, 3
NCORE = 8
BC = B // NCORE          # 128 batch rows per core
KT = H // 128            # 4 hidden k-tiles
GT = (3 * H) // 128      # 12 gate tiles (r 0-3, z 4-7, n 8-11)


# ----------------------------------------------------------------- host packing

def _pack_st(W, nk, ng):
    """Stationary pack of W [ng*128, nk*128] -> [128, nk*ng*128] fp16.

    Column block (k*ng + g)*128 holds lhsT for (ktile k, gtile g):
    lhsT[kf, gc] = W[g*128+gc, k*128+kf]. (phase-1 gemm only)
    """
    t = W.reshape(ng, 128, nk, 128).transpose(2, 3, 0, 1)  # [k, kf, g, gc]
    return np.ascontiguousarray(
        t.transpose(1, 0, 2, 3).reshape(128, nk * ng * 128)
    ).astype(np.float16)


def _pack_whhT(W):
    """Moving-operand pack of Whh [3H, H] -> [128, KT*3H] fp16.

    Column block k*1536..(k+1)*1536 = Whh[:, k*128:(k+1)*128].T, i.e.
    [kf, g] with g ordered r(0:512) z(512:1024) n(1024:1536).
    """
    out = np.zeros((128, KT * 3 * H), np.float32)
    for k in range(KT):
        out[:, k * 3 * H:(k + 1) * 3 * H] = W[:, k * 128:(k + 1) * 128].T
    return out.astype(np.float16)


def _pack_wihbT(Wih, bih, bhh):
    """x-stationary moving pack: [4, 2048] fp16.

    Rows 0-2 = Wih^T (IN=3 input feats), row 3 = bias.
    Cols: r(0:512)=Wih_r^T,b_r sum | z | ni(1024:1536)=Wih_n^T,bih_n |
    nh(1536:2048)= zeros, bhh_n.
    """
    out = np.zeros((4, 4 * H), np.float32)
    out[0:IN, 0:512] = Wih[0:512, :].T
    out[0:IN, 512:1024] = Wih[512:1024, :].T
    out[0:IN, 1024:1536] = Wih[1024:1536, :].T
    bsum = bih + bhh
    out[3, 0:512] = bsum[0:512]
    out[3, 512:1024] = bsum[512:1024]
    out[3, 1024:1536] = bih[1024:1536]
    out[3, 1536:2048] = bhh[1024:1536]
    return out.astype(np.float16)


def _pack_wih_hT(Wih):
    """hidden-input moving pack for decoder layers 1-3: [128, KT*3H] fp16."""
    return _pack_whhT(Wih)


def _bias_rows(bih, bhh):
    """[1, 2048] bias row: r,z sums; ni=bih_n; nh=bhh_n."""
    row = np.zeros((1, 4 * H), np.float32)
    bsum = bih + bhh
    row[0, 0:512] = bsum[0:512]
    row[0, 512:1024] = bsum[512:1024]
    row[0, 1024:1536] = bih[1024:1536]
    row[0, 1536:2048] = bhh[1024:1536]
    return row.astype(np.float16)


def _host_pack(inp, s_steps, t_steps):
    """Build replicated weight arrays + per-core input arrays."""
    g = lambda k: np.asarray(inp[k], np.float32)
    w = {}
    for d, tag in ((0, "f"), (1, "b")):
        w[f"w_hhT0{tag}"] = _pack_whhT(g("enc_Whh0")[d])
        w[f"w_ihbT0{tag}"] = _pack_wihbT(
            g("enc_Wih0")[d], g("enc_bih0")[d], g("enc_bhh0")[d])
        w[f"w_hhT1{tag}"] = _pack_whhT(g("enc_Whh1")[d])
        w[f"b_row1{tag}"] = _bias_rows(g("enc_bih1")[d], g("enc_bhh1")[d])
    # encoder layer 1 input gemm (phase 1, weight-stationary): K = 2H, G = 2*GT.
    # Mixed precision: fwd-direction k-tiles (0-3) fp8 DoubleRow (weights
    # x16, activations x16), bwd k-tiles (4-7) fp16 with weights x256 so both
    # halves share the psum scale; the output copy divides by 256.
    w_all = np.concatenate([g("enc_Wih1")[0], g("enc_Wih1")[1]], axis=0)  # [2*3H, 2H]
    wp = _pack_st(w_all, 2 * KT, 2 * GT).astype(np.float32)  # [128, 8*2GT*128]
    wp3 = wp.reshape(128, 2 * KT, 2 * GT * 128)
    w["w_ih1"] = np.ascontiguousarray(
        wp3[:, 0:KT] * W8SCALE).astype(NP8)  # [128, 4, 2GT*128]
    w["w_ih1b"] = np.ascontiguousarray(
        wp3[:, KT:2 * KT] * (W8SCALE * W8SCALE)).astype(np.float16)
    bg = np.zeros((128, 2 * GT), np.float32)
    for d in range(2):
        bsum = g("enc_bih1")[d] + g("enc_bhh1")[d]
        for j in range(8):
            bg[:, d * GT + j] = bsum[j * 128:(j + 1) * 128]
        for j in range(4):
            bg[:, d * GT + 8 + j] = g("enc_bih1")[d][1024 + j * 128:1024 + (j + 1) * 128]
    w["b_gemm"] = bg
    w["ident"] = np.eye(128, dtype=np.float16)
    # decoder
    w["w_dihbT0"] = _pack_wihbT(g("dec_Wih0"), g("dec_bih0"), g("dec_bhh0"))
    w["w_dhhT"] = np.concatenate(
        [_pack_whhT(g("dec_Whh0"))]
        + [_pack_whhT(g("dec_Whh123")[i]) for i in range(3)], axis=1)
    w["w_dihT"] = np.concatenate(
        [_pack_wih_hT(g("dec_Wih123")[i]) for i in range(3)], axis=1)
    w["w_dbias"] = np.concatenate(
        [_bias_rows(g("dec_bih123")[i], g("dec_bhh123")[i]) for i in range(3)],
        axis=1)  # [1, 3*2048]
    lw = np.zeros((128, KT), np.float32)
    for k in range(KT):
        lw[:, k] = g("lin_W")[0, k * 128:(k + 1) * 128]
    w["w_lin"] = lw.astype(np.float16)
    lin_b = float(np.asarray(inp["lin_b"]).reshape(-1)[0])

    x = np.asarray(inp["x"], np.float32)  # [B, S, IN]
    per_core = []
    for c in range(NCORE):
        xc = x[c * BC:(c + 1) * BC]                      # [BC, S, IN]
        xt = xc.transpose(2, 1, 0)[:, :s_steps, :]       # [IN, s, BC]
        xa = np.zeros((4, s_steps, BC), np.float32)
        xa[0:IN] = xt
        xa[IN] = 1.0
        m = dict(w)
        m["xa"] = np.ascontiguousarray(xa.reshape(4, s_steps * BC)).astype(np.float16)
        xlast = xc[:, s_steps - 1, :].T                  # [IN, BC]
        m["xd0"] = xlast.astype(np.float16)
        m["xd32"] = np.ascontiguousarray(xlast[0:2].T).astype(np.float32)  # [BC, 2]
        per_core.append(m)
    return per_core, lin_b


# ----------------------------------------------------------------- device build

def build_nc(s_steps=S, t_steps=T, lin_b=0.0, repeat=1):
    nc = bacc.Bacc("TRN2", target_bir_lowering=False, debug=False,
                   num_devices=NCORE)

    dram_in = {}
    for name, shape, dt in [
        ("w_hhT0f", [128, KT * 3 * H], FP16), ("w_hhT0b", [128, KT * 3 * H], FP16),
        ("w_ihbT0f", [4, 4 * H], FP16), ("w_ihbT0b", [4, 4 * H], FP16),
        ("w_hhT1f", [128, KT * 3 * H], FP16), ("w_hhT1b", [128, KT * 3 * H], FP16),
        ("b_row1f", [1, 4 * H], FP16), ("b_row1b", [1, 4 * H], FP16),
        ("w_ih1", [128, KT, 2 * GT * 128], FP8),
        ("w_ih1b", [128, KT, 2 * GT * 128], FP16),
        ("b_gemm", [128, 2 * GT], FP32),
        ("ident", [128, 128], FP16),
        ("w_dihbT0", [4, 4 * H], FP16),
        ("w_dhhT", [128, 4 * KT * 3 * H], FP16),
        ("w_dihT", [128, 3 * KT * 3 * H], FP16),
        ("w_dbias", [1, 3 * 4 * H], FP16),
        ("w_lin", [128, KT], FP16),
        ("xa", [4, s_steps * BC], FP16),
        ("xd0", [3, BC], FP16),
        ("xd32", [BC, 2], FP32),
    ]:
        dram_in[name] = nc.dram_tensor(name, shape, dt, kind="ExternalInput").ap()

    y_dram = nc.dram_tensor("y", [BC, t_steps], FP32, kind="ExternalOutput").ap()

    NCH = s_steps // 4  # gemm chunks (4 timesteps each)

    with tile.TileContext(nc) as tc, ExitStack() as ctx:
        const = ctx.enter_context(tc.tile_pool(name="const", bufs=1))
        hp = ctx.enter_context(tc.tile_pool(name="hstate", bufs=2))
        dram = ctx.enter_context(tc.tile_pool(name="dram", bufs=1, space="DRAM"))

        ones1 = const.tile([1, BC], FP16, tag="ones1")
        nc.vector.memset(ones1[:], 1.0)
        ones512 = const.tile([128, 512], FP16, tag="ones512")
        nc.vector.memset(ones512[:], 1.0)
        linb_sb = const.tile([1, 1], FP32, tag="linb_sb")
        nc.vector.memset(linb_sb[:], float(lin_b))
        y_sb = const.tile([128, t_steps], FP32, tag="y_sb")

        out0_f8 = dram.tile([s_steps, 128, KT * 128], FP8, tag="out0_0",
                            name="out0_0")
        out0_16 = dram.tile([s_steps, 128, KT * 128], FP16, tag="out0_1",
                            name="out0_1")
        gi1_d = [dram.tile([s_steps, 128, GT * 128], FP16, tag=f"gi1_{d}",
                           name=f"gi1_{d}") for d in range(2)]

        def load_const(name, pool):
            ap = dram_in[name]
            t = pool.tile(list(ap.shape), ap.dtype, tag=name)
            nc.sync.dma_start(t[:], ap[:])
            return t

        def gates_bf(nc_, gw, pr, pz, pni, pnh, h_bf_prev, tag, htag=None):
            """[batch, gate]-layout gate math; returns new h_bf (fp16).

            h' = (1-z)*n + z*h; z*h and (1-z) are computed while the tanh
            runs so only two DVE ops trail the tanh on the critical chain.
            """
            htag = htag or tag
            r16 = gw.tile([128, 512], FP16, tag=f"r{tag}")
            nc_.scalar.activation(r16[:], pr[:], AF.Sigmoid)
            z16 = gw.tile([128, 512], FP16, tag=f"z{tag}")
            nc_.scalar.activation(z16[:], pz[:], AF.Sigmoid)
            t2 = gw.tile([128, 512], FP16, tag=f"t2{tag}")
            nc_.vector.tensor_mul(t2[:], r16[:], pnh[:])
            t3 = gw.tile([128, 512], FP16, tag=f"t3{tag}")
            nc_.vector.tensor_add(t3[:], t2[:], pni[:])
            n16 = gw.tile([128, 512], FP16, tag=f"n{tag}")
            nc_.scalar.activation(n16[:], t3[:], AF.Tanh)
            # q = z*h and oz = 1-z run on the idle gpsimd engine while the
            # tanh executes; the post-tanh tail is halved across DVE+gpsimd
            # so only ~2 half-width ops trail the tanh on the critical chain.
            q16 = gw.tile([128, 512], FP16, tag=f"q{tag}")
            nc_.vector.tensor_mul(q16[:], z16[:], h_bf_prev[:])
            oz16 = gw.tile([128, 512], FP16, tag=f"oz{tag}")
            nc_.vector.tensor_sub(oz16[:], ones512[:], z16[:])
            m16 = gw.tile([128, 512], FP16, tag=f"m{tag}")
            nc_.vector.tensor_mul(m16[:], oz16[:], n16[:])
            h_bf = hp.tile([128, 512], FP16, tag=f"hbf{htag}")
            nc_.vector.tensor_add(h_bf[:], m16[:], q16[:])
            return h_bf

        def transpose_h(nc_, psp, ident, h_bf, tag, bufs=1, fp8_pool=None):
            """4 identity matmuls: h_bf [b, feat] -> psum [feat, b] slices."""
            pT = psp.tile([128, 512], FP32, tag="pT", bufs=bufs)
            for k in range(KT):
                nc_.tensor.matmul(pT[:, k * 128:(k + 1) * 128],
                                  h_bf[:, k * 128:(k + 1) * 128], ident[:],
                                  start=(k == 0), stop=(k == KT - 1))
            # psum->sbuf copy halved across scalar+DVE; ktile 0 lands first
            # so the next step's first stationary load isn't blocked
            h_fb = hp.tile([128, 512], FP16, tag=f"hfb{tag}")
            nc_.scalar.activation(h_fb[:, 0:256], pT[:, 0:256], AF.Identity)
            nc_.vector.tensor_copy(h_fb[:, 256:512], pT[:, 256:512])
            if fp8_pool is None:
                return h_fb
            h8 = fp8_pool.tile([128, 512], FP8, tag=f"h8{tag}")
            nc_.scalar.activation(h8[:], pT[:], AF.Identity, scale=W8SCALE)
            return h_fb, h8

        def scan_mms(nc_, psp, wh, h_fb_d):
            """hh matmuls for one scan step-dir: pr/pz k-major first
            (single-buffered — their WAR frees earliest via the sigmoids),
            pnh k-loop last; pnh/pni double-buffered."""
            pr = psp.tile([128, 512], FP32, tag="pr", bufs=1)
            pz = psp.tile([128, 512], FP32, tag="pz", bufs=1)
            for k in range(KT):
                hk = h_fb_d[:, k * 128:(k + 1) * 128]
                st = (k == 0)
                nc_.tensor.matmul(pr[:], hk, wh[:, k * 1536:k * 1536 + 512],
                                  start=st, stop=False)
                nc_.tensor.matmul(pz[:], hk,
                                  wh[:, k * 1536 + 512:k * 1536 + 1024],
                                  start=st, stop=False)
            pnh = psp.tile([128, 512], FP32, tag="pnh", bufs=2)
            for k in range(KT):
                hk = h_fb_d[:, k * 128:(k + 1) * 128]
                nc_.tensor.matmul(pnh[:], hk,
                                  wh[:, k * 1536 + 1024:k * 1536 + 1536],
                                  start=(k == 0), stop=False)
            pni = psp.tile([128, 512], FP32, tag="pni", bufs=2)
            return pr, pz, pni, pnh

        NPH = int(os.environ.get("BGRU_PHASES", "4"))

        def emit_phases(sfx):
          # =================== phase 0: encoder layer-0 bidirectional scan
          if NPH >= 1:
            with tc.tile_pool(name="p0w" + sfx, bufs=1) as p0w, \
                 tc.tile_pool(name="gw0" + sfx, bufs=2) as gw0, \
                 tc.tile_pool(name="psum0" + sfx, bufs=1, space="PSUM") as ps0:
              whhT = [load_const("w_hhT0f", p0w), load_const("w_hhT0b", p0w)]
              wihbT = [load_const("w_ihbT0f", p0w), load_const("w_ihbT0b", p0w)]
              xa = load_const("xa", p0w)
              ident = load_const("ident", p0w)

              h_bf = [None, None]
              h_fb = [None, None]
              for d in range(2):
                  h_bf[d] = hp.tile([128, 512], FP16, tag=f"hbf_e0{d}",
                                    name=f"hbf_e0{d}")
                  nc.vector.memset(h_bf[d][:], 0.0)
                  h_fb[d] = hp.tile([128, 512], FP16, tag=f"hfb_e0{d}",
                                    name=f"hfb_e0{d}")
                  nc.vector.memset(h_fb[d][:], 0.0)

              def p0_mm(d, step):
                  tt = step if d == 0 else s_steps - 1 - step
                  # h-independent ih/bias matmuls OPEN each bank group so the
                  # block starts without waiting on the new h's transpose copy
                  xst = xa[0:4, tt * BC:(tt + 1) * BC]
                  wi = wihbT[d]
                  wh = whhT[d]
                  pr = ps0.tile([128, 512], FP32, tag="pr", bufs=1)
                  pz = ps0.tile([128, 512], FP32, tag="pz", bufs=1)
                  pnh = ps0.tile([128, 512], FP32, tag="pnh", bufs=2)
                  pni = ps0.tile([128, 512], FP32, tag="pni", bufs=2)
                  nc.tensor.matmul(pr[:], xst, wi[0:4, 0:512],
                                   start=True, stop=False)
                  nc.tensor.matmul(pz[:], xst, wi[0:4, 512:1024],
                                   start=True, stop=False)
                  nc.tensor.matmul(pnh[:], xst, wi[0:4, 1536:2048],
                                   start=True, stop=False)
                  nc.tensor.matmul(pni[:], xst, wi[0:4, 1024:1536],
                                   start=True, stop=True)
                  for k in range(KT):
                      hk = h_fb[d][:, k * 128:(k + 1) * 128]
                      nc.tensor.matmul(pr[:], hk, wh[:, k * 1536:k * 1536 + 512],
                                       start=False, stop=(k == KT - 1))
                  for k in range(KT):
                      hk = h_fb[d][:, k * 128:(k + 1) * 128]
                      nc.tensor.matmul(pnh[:], hk,
                                       wh[:, k * 1536 + 1024:k * 1536 + 1536],
                                       start=False, stop=(k == KT - 1))
                  for k in range(KT):
                      hk = h_fb[d][:, k * 128:(k + 1) * 128]
                      nc.tensor.matmul(pz[:], hk,
                                       wh[:, k * 1536 + 512:k * 1536 + 1024],
                                       start=False, stop=(k == KT - 1))
                  return (pr, pz, pni, pnh)

              def p0_gates(d, step, ps):
                  tt = step if d == 0 else s_steps - 1 - step
                  h_bf[d] = gates_bf(nc, gw0, *ps, h_bf[d], f"e0{d}")
                  if d == 0:
                      h_fb[d], h8 = transpose_h(nc, ps0, ident, h_bf[d],
                                                f"e0{d}", bufs=2, fp8_pool=gw0)
                      nc.sync.dma_start(out0_f8[tt], h8[:])
                  else:
                      h_fb[d] = transpose_h(nc, ps0, ident, h_bf[d], f"e0{d}",
                                            bufs=2)
                      nc.sync.dma_start(out0_16[tt], h_fb[d][:])

              # dir-skewed pipeline: gates/T of d1(s-1) sit between d0(s)'s
              # and d1(s)'s matmul blocks so PE never waits on a gate chain.
              ps_cur = [None, None]
              for step in range(s_steps):
                  ps_cur[0] = p0_mm(0, step)
                  if step > 0:
                      p0_gates(1, step - 1, ps_cur[1])
                  ps_cur[1] = p0_mm(1, step)
                  p0_gates(0, step, ps_cur[0])
              p0_gates(1, s_steps - 1, ps_cur[1])
              h_dec0 = [h_fb[0], h_fb[1]]  # (h0f, h0b) [feat, b]
              h_dec0_bf = [h_bf[0], h_bf[1]]

          # =================== phase 1: in1 @ Wih1.T big gemm (4 timesteps/chunk)
          if NPH >= 2:
            with tc.tile_pool(name="p1w" + sfx, bufs=1) as p1w, \
                 tc.tile_pool(name="gmv" + sfx, bufs=3) as gmv, \
                 tc.tile_pool(name="gst" + sfx, bufs=6) as gst, \
                 tc.tile_pool(name="psumg" + sfx, bufs=8, space="PSUM") as psg:
              wih1 = load_const("w_ih1", p1w)
              wih1b = load_const("w_ih1b", p1w)
              bgem = load_const("b_gemm", p1w)

              order = []
              lo, hi = 0, NCH - 1
              while lo <= hi:
                  order.append(lo)
                  if hi != lo:
                      order.append(hi)
                  lo, hi = lo + 1, hi - 1

              for c in order:
                  mv8 = gmv.tile([128, KT, 512], FP8, tag="mv8")
                  mv16 = []
                  for j in range(KT):
                      src = out0_f8[4 * c:4 * c + 4, :, j * 128:(j + 1) * 128]
                      nc.sync.dma_start(
                          mv8[:, j, :].rearrange("p (t b) -> p t b", t=4),
                          src.rearrange("t p b -> p t b"))
                  for j in range(KT):
                      m = gmv.tile([128, 512], FP16, tag=f"mv16{j}")
                      src = out0_16[4 * c:4 * c + 4, :, j * 128:(j + 1) * 128]
                      nc.sync.dma_start(
                          m[:].rearrange("p (t b) -> p t b", t=4),
                          src.rearrange("t p b -> p t b"))
                      mv16.append(m)
                  for G in range(2 * GT):
                      q = psg.tile([128, 512], FP32, tag="gq")
                      for m in range(KT // 2):
                          nc.tensor.matmul(
                              q[:],
                              wih1[:, 2 * m:2 * m + 2, G * 128:(G + 1) * 128],
                              mv8[:, 2 * m:2 * m + 2, :],
                              start=(m == 0), stop=False,
                              perf_mode=mybir.MatmulPerfMode.DoubleRow)
                      for j in range(KT):
                          nc.tensor.matmul(
                              q[:],
                              wih1b[:, j, G * 128:(G + 1) * 128],
                              mv16[j][:], start=False, stop=(j == KT - 1))
                      gsb = gst.tile([128, 512], FP16, tag="gsb")
                      nc.scalar.activation(gsb[:], q[:], AF.Identity,
                                           bias=bgem[:, G:G + 1],
                                           scale=1.0 / (W8SCALE * W8SCALE))
                      d_, gl = (0, G) if G < GT else (1, G - GT)
                      dst = gi1_d[d_][4 * c:4 * c + 4, :, gl * 128:(gl + 1) * 128]
                      nc.sync.dma_start(dst.rearrange("t p b -> p t b"),
                                        gsb[:].rearrange("p (t b) -> p t b", t=4))

          # =================== phase 2: encoder layer-1 bidirectional scan
          if NPH >= 3:
            with tc.tile_pool(name="p2w" + sfx, bufs=1) as p2w, \
                 tc.tile_pool(name="gw2" + sfx, bufs=2) as gw2, \
                 tc.tile_pool(name="gild" + sfx, bufs=3) as gild, \
                 tc.tile_pool(name="psum1" + sfx, bufs=1, space="PSUM") as ps1:
              whhT1 = [load_const("w_hhT1f", p2w), load_const("w_hhT1b", p2w)]
              brow1 = [load_const("b_row1f", p2w), load_const("b_row1b", p2w)]
              ident = load_const("ident", p2w)

              h_bf = [None, None]
              h_fb = [None, None]
              for d in range(2):
                  h_bf[d] = hp.tile([128, 512], FP16, tag=f"hbf_e1{d}",
                                    name=f"hbf_e1{d}")
                  nc.vector.memset(h_bf[d][:], 0.0)
                  h_fb[d] = hp.tile([128, 512], FP16, tag=f"hfb_e1{d}",
                                    name=f"hfb_e1{d}")
                  nc.vector.memset(h_fb[d][:], 0.0)

              def p2_mm(d, step):
                  tt = step if d == 0 else s_steps - 1 - step
                  gld = gild.tile([128, GT * 128], FP16, tag=f"gi{d}")
                  nc.sync.dma_start(gld[:], gi1_d[d][tt])
                  wh = whhT1[d]
                  pr = ps1.tile([128, 512], FP32, tag="pr", bufs=1)
                  pz = ps1.tile([128, 512], FP32, tag="pz", bufs=1)
                  pnh = ps1.tile([128, 512], FP32, tag="pnh", bufs=2)
                  pni = ps1.tile([128, 512], FP32, tag="pni", bufs=2)
                  # h-independent openers: gi1 injections + nh bias
                  for m in range(4):
                      nc.tensor.matmul(pr[:, m * 128:(m + 1) * 128],
                                       gld[:, m * 128:(m + 1) * 128], ident[:],
                                       start=(m == 0), stop=False)
                  for m in range(4):
                      nc.tensor.matmul(pz[:, m * 128:(m + 1) * 128],
                                       gld[:, (4 + m) * 128:(5 + m) * 128],
                                       ident[:], start=(m == 0), stop=False)
                  nc.tensor.matmul(pnh[:], ones1[:], brow1[d][0:1, 1536:2048],
                                   start=True, stop=False)
                  for m in range(4):
                      nc.tensor.matmul(pni[:, m * 128:(m + 1) * 128],
                                       gld[:, (8 + m) * 128:(9 + m) * 128],
                                       ident[:], start=(m == 0), stop=(m == 3))
                  for k in range(KT):
                      hk = h_fb[d][:, k * 128:(k + 1) * 128]
                      nc.tensor.matmul(pr[:], hk, wh[:, k * 1536:k * 1536 + 512],
                                       start=False, stop=(k == KT - 1))
                  for k in range(KT):
                      hk = h_fb[d][:, k * 128:(k + 1) * 128]
                      nc.tensor.matmul(pnh[:], hk,
                                       wh[:, k * 1536 + 1024:k * 1536 + 1536],
                                       start=False, stop=(k == KT - 1))
                  for k in range(KT):
                      hk = h_fb[d][:, k * 128:(k + 1) * 128]
                      nc.tensor.matmul(pz[:], hk,
                                       wh[:, k * 1536 + 512:k * 1536 + 1024],
                                       start=False, stop=(k == KT - 1))
                  return (pr, pz, pni, pnh)

              def p2_gates(d, ps):
                  h_bf[d] = gates_bf(nc, gw2, *ps, h_bf[d], f"e1{d}")
                  h_fb[d] = transpose_h(nc, ps1, ident, h_bf[d], f"e1{d}",
                                        bufs=2)

              ps_cur = [None, None]
              for step in range(s_steps):
                  ps_cur[0] = p2_mm(0, step)
                  if step > 0:
                      p2_gates(1, ps_cur[1])
                  ps_cur[1] = p2_mm(1, step)
                  p2_gates(0, ps_cur[0])
              p2_gates(1, ps_cur[1])
              h_dec1 = [h_fb[0], h_fb[1]]
              h_dec1_bf = [h_bf[0], h_bf[1]]

          # =================== phase 3: 4-layer decoder, autoregressive
          if NPH >= 4:
            with tc.tile_pool(name="p3w" + sfx, bufs=1) as p3w, \
                 tc.tile_pool(name="gw3" + sfx, bufs=2) as gw3, \
                 tc.tile_pool(name="dx" + sfx, bufs=2) as dx, \
                 tc.tile_pool(name="psumd" + sfx, bufs=1, space="PSUM") as psd:
              wdhhT = load_const("w_dhhT", p3w)
              wdihT = load_const("w_dihT", p3w)
              wdbias = load_const("w_dbias", p3w)
              wdihbT0 = load_const("w_dihbT0", p3w)
              wlin = load_const("w_lin", p3w)
              ident = load_const("ident", p3w)

              # x_aug stationary [4, BC]: rows x0,x1,x2,ones
              xst = dx.tile([4, BC], FP16, tag="xst", bufs=2)
              nc.sync.dma_start(xst[0:3, :], dram_in["xd0"][:])
              nc.sync.dma_start(xst[3:4, :], ones1[:])
              d01 = dx.tile([128, 2], FP32, tag="d01", bufs=1)
              nc.sync.dma_start(d01[:], dram_in["xd32"][:])
              din = [d01[:, 0:1], d01[:, 1:2]]

              hfbL = [h_dec0[0], h_dec0[1], h_dec1[0], h_dec1[1]]
              hbfL = [h_dec0_bf[0], h_dec0_bf[1], h_dec1_bf[0], h_dec1_bf[1]]

              def open_hh(L):
                  """Alloc pr/pz/pnh banks, emit hh matmuls (no stop)."""
                  pr = psd.tile([128, 512], FP32, tag="pr", bufs=2)
                  pz = psd.tile([128, 512], FP32, tag="pz", bufs=2)
                  wh = wdhhT[:, L * KT * 1536:]
                  for k in range(KT):
                      hk = hfbL[L][:, k * 128:(k + 1) * 128]
                      st = (k == 0)
                      nc.tensor.matmul(pr[:], hk, wh[:, k * 1536:k * 1536 + 512],
                                       start=st, stop=False)
                      nc.tensor.matmul(pz[:], hk,
                                       wh[:, k * 1536 + 512:k * 1536 + 1024],
                                       start=st, stop=False)
                  pnh = psd.tile([128, 512], FP32, tag="pnh", bufs=2)
                  for k in range(KT):
                      hk = hfbL[L][:, k * 128:(k + 1) * 128]
                      nc.tensor.matmul(pnh[:], hk,
                                       wh[:, k * 1536 + 1024:k * 1536 + 1536],
                                       start=(k == 0), stop=False)
                  return [pr, pz, None, pnh]

              def close_ih(L, ps, below_fb):
                  pr, pz, _, pnh = ps
                  pni = psd.tile([128, 512], FP32, tag="pni", bufs=1)
                  ps[2] = pni
                  if L == 0:
                      wi = wdihbT0
                      nc.tensor.matmul(pr[:], xst[0:4, :], wi[0:4, 0:512],
                                       start=False, stop=True)
                      nc.tensor.matmul(pz[:], xst[0:4, :], wi[0:4, 512:1024],
                                       start=False, stop=True)
                      nc.tensor.matmul(pnh[:], xst[0:4, :], wi[0:4, 1536:2048],
                                       start=False, stop=True)
                      nc.tensor.matmul(pni[:], xst[0:4, :], wi[0:4, 1024:1536],
                                       start=True, stop=True)
                  else:
                      # below-independent bias openers first, then the
                      # below_fb-dependent k-loops
                      wi = wdihT[:, (L - 1) * KT * 1536:]
                      br = wdbias[0:1, (L - 1) * 2048:]
                      nc.tensor.matmul(pr[:], ones1[:], br[0:1, 0:512],
                                       start=False, stop=False)
                      nc.tensor.matmul(pz[:], ones1[:], br[0:1, 512:1024],
                                       start=False, stop=False)
                      nc.tensor.matmul(pnh[:], ones1[:], br[0:1, 1536:2048],
                                       start=False, stop=True)
                      nc.tensor.matmul(pni[:], ones1[:], br[0:1, 1024:1536],
                                       start=True, stop=False)
                      for k in range(KT):
                          bk = below_fb[:, k * 128:(k + 1) * 128]
                          nc.tensor.matmul(pr[:], bk, wi[:, k * 1536:k * 1536 + 512],
                                           start=False, stop=(k == KT - 1))
                      for k in range(KT):
                          bk = below_fb[:, k * 128:(k + 1) * 128]
                          nc.tensor.matmul(pni[:], bk,
                                           wi[:, k * 1536 + 1024:k * 1536 + 1536],
                                           start=False, stop=(k == KT - 1))
                      for k in range(KT):
                          bk = below_fb[:, k * 128:(k + 1) * 128]
                          nc.tensor.matmul(pz[:], bk,
                                           wi[:, k * 1536 + 512:k * 1536 + 1024],
                                           start=False, stop=(k == KT - 1))

              xnew_prev = None
              for t in range(t_steps):
                  # software-pipelined: hh matmuls of layers L+1/L+2 are
                  # emitted before layer L's input-dependent matmuls.
                  ps_of = {0: open_hh(0), 1: open_hh(1)}
                  if t > 0:
                      # feedback transpose deferred here so the hh matmuls
                      # above cover the feedback chain's latency
                      pT3 = psd.tile([128, 512], FP32, tag="pT", bufs=1)
                      pxT = pT3[0:4, 0:BC]
                      nc.tensor.matmul(pxT, xnew_prev[:, 0:4], ident[:],
                                       start=True, stop=True)
                      xst = dx.tile([4, BC], FP16, tag="xst", bufs=2)
                      nc.scalar.activation(xst[:], pxT, AF.Identity)
                  below_fb = None
                  for L in range(4):
                      close_ih(L, ps_of[L], below_fb)
                      if L + 2 < 4:
                          ps_of[L + 2] = open_hh(L + 2)
                      hbfL[L] = gates_bf(nc, gw3, *ps_of[L], hbfL[L], "dd", htag=f"d{L}")
                      hfbL[L] = transpose_h(nc, psd, ident, hbfL[L], f"d{L}",
                                            bufs=1)
                      below_fb = hfbL[L]

                  # output projection + feedback ([b, 1] column layout)
                  pT2 = psd.tile([128, 512], FP32, tag="pT", bufs=1)
                  pout = pT2[:, 0:1]
                  for k in range(KT):
                      nc.tensor.matmul(pout,
                                       below_fb[:, k * 128:(k + 1) * 128],
                                       wlin[:, k:k + 1],
                                       start=(k == 0), stop=(k == KT - 1))
                  ysl = y_sb[:, t:t + 1]
                  nc.vector.tensor_scalar_add(ysl, pout, float(lin_b))
                  if t + 1 < t_steps:
                      xnew = dx.tile([128, 4], FP16, tag="xnew")
                      s0 = dx.tile([128, 1], FP32, tag="s0")
                      nc.vector.tensor_copy(s0[:], ysl)
                      s1 = dx.tile([128, 1], FP32, tag="s1")
                      nc.vector.tensor_sub(s1[:], din[0], s0[:])
                      s2 = dx.tile([128, 1], FP32, tag="s2")
                      nc.vector.tensor_sub(s2[:], din[1], s1[:])
                      nc.vector.tensor_copy(xnew[:, 0:1], s0[:])
                      nc.vector.tensor_copy(xnew[:, 1:2], s1[:])
                      nc.vector.tensor_copy(xnew[:, 2:3], s2[:])
                      nc.vector.memset(xnew[:, 3:4], 1.0)
                      xnew_prev = xnew
                      din = [s0[:], s1[:]]

              nc.sync.dma_start(y_dram[:], y_sb[:, 0:t_steps])

        for _rep in range(repeat):
            emit_phases(f"_{_rep}" if _rep else "")

    nc.compile()
    return nc


# ----------------------------------------------------------------- entry point

def kernel(**inputs) -> np.ndarray:
    s_steps = int(os.environ.get("BGRU_S", S))
    t_steps = int(os.environ.get("BGRU_T", 0)) or int(inputs.get("target_len", T))
    per_core, lin_b = _host_pack(inputs, s_steps, t_steps)
    nc = build_nc(s_steps, t_steps, lin_b)
    res = run_bass_kernel_spmd(nc, per_core, list(range(NCORE)))
    out = np.zeros((B, t_steps, 1), np.float32)
    for c in range(NCORE):
        yc = res.results[c]["y"].reshape(BC, t_steps)  # [b, t]
        out[c * BC:(c + 1) * BC, :, 0] = yc
    return out
